# revision 12
# baseline (speedup 1.0000x reference)
"""Trainium2 Bass kernel for nn_CNNModel (gnn_message_passing).

Model: 3 sparse grouped 3x3 convs (fan-in 3/16/32, SAME pad, relu) on
[64,32,32,3] -> flatten -> dense(131072,50)+relu -> dense(50,10) -> softmax.

Sharding (8 cores): spatial over H. Core k computes L2 output rows
[4k, 4k+4) for ALL 64 images (halo rows recomputed per core: L1 rows
[4k-1,4k+5), L0 rows [4k-2,4k+6)), then the dense1 partial product over
its 16384 flattened features, a tiny [50,64] AllReduce, and a replicated
dense2+softmax tail.

v2 (vs baseline at 234us):
  - all-fp16 datapath (p0/w0 fp16 with -60000 poison; dw1/l2 fp16) so
    L0 and dense1 leave the fp32 quarter-rate PE path.
  - packed constants in 2 DMAs + triggers spread over sync/scalar/gpsimd
    (baseline serialized 33 DIRECT2D triggers on sync; first MM at 29us).
  - L0->L1 pipelined per 8-image group with relu work split across
    ACT (L0), GPSIMD tensor_scalar add+max (L1), DVE (shift copies) so
    the PE never stalls long enough for HAM to re-throttle (baseline ran
    ALL of L1 at the 1.2GHz cold clock).
  - L1 out-of-image halo rows zeroed via per-core poison bias
    (b1 - 1e9 on the affected core) instead of If(partition_id) memsets.
  - dense1 as 64 stacked [128,100]x[128,128] fp16 matmuls into a
    [100,128] psum with diagonal extraction (baseline: 256 fp32 N=64
    quarter-rate matmuls).
  - short softmax tail: no max-subtract, exp+sum fused via accum_out.
"""
import os
import sys

if "/opt/trn_rl_repo" not in sys.path:
    sys.path.insert(0, "/opt/trn_rl_repo")

os.environ.setdefault("JAX_COMPILATION_CACHE_DIR", "/tmp/jax_comp_cache")
os.environ.setdefault("JAX_PERSISTENT_CACHE_MIN_COMPILE_TIME_SECS", "1")
os.environ.setdefault("JAX_PERSISTENT_CACHE_MIN_ENTRY_SIZE_BYTES", "0")

from contextlib import ExitStack

import numpy as np

NCORES = 8
B, H, W = 64, 32, 32
NEG = -1.0e9
NEGH = -60000.0  # poison weight, must stay fp16-representable

_CACHE = {}


def _make_idx():
    I = np.eye(16)
    w1 = sum(np.roll(I, shift=j, axis=0) for j in range(4))
    w2 = sum(np.roll(I, shift=4 * j, axis=0) for j in range(4))
    conn1 = np.kron(np.ones((8, 4)), w1)  # [128, 64]
    conn2 = np.kron(np.ones((8, 8)), w2)  # [128, 128]
    idx1 = np.stack([np.nonzero(conn1[r])[0] for r in range(128)])
    idx2 = np.stack([np.nonzero(conn2[r])[0] for r in range(128)])
    return idx1, idx2


def _dense_w(cw, idx, cin):
    # cw [3,3,F,n], idx [n,F] -> dense [3,3,cin,n]
    n, _ = idx.shape
    wd = np.zeros((3, 3, cin, n), np.float32)
    for node in range(n):
        wd[:, :, idx[node], node] = cw[:, :, :, node]
    return wd


def _build_p0(inputs, k):
    """Host im2col pack for core k: [28, B*8*32] fp16.

    Row (ky*3+kx)*3+c at (b, r, x) = input[b, (4k-2+r)+ky-1, x+kx-1, c]
    (zero-padded). Row 27 = 1.0 on out-of-image L0 rows (poison indicator).
    """
    p = np.zeros((28, B, 8, 32), np.float32)
    xpad = np.zeros((B, H + 8, W + 2, 3), np.float32)
    xpad[:, 4 : 4 + H, 1 : 1 + W, :] = inputs
    for ky in range(3):
        for kx in range(3):
            for c in range(3):
                row = (ky * 3 + kx) * 3 + c
                g0 = 4 * k - 2 + ky - 1 + 4  # padded row index for r=0
                p[row] = xpad[:, g0 : g0 + 8, kx : kx + 32, c]
    for r in range(8):
        g = 4 * k - 2 + r
        if g < 0 or g >= H:
            p[27, :, r, :] = 1.0
    return np.ascontiguousarray(p.reshape(28, -1)).astype(np.float16)


def _pack_dw1(dw1, k):
    # core k's dw1 rows -> [c, m*128 + kk*64 + j], j>=50 zero-padded
    ds = (
        dw1[k * 16384 : (k + 1) * 16384]
        .reshape(64, 2, 128, 50)  # [pair m, kk, c, j]
        .transpose(2, 0, 1, 3)  # [c, m, kk, j]
    )
    pack = np.zeros((128, 64, 2, 64), np.float16)
    pack[:, :, :, 0:50] = ds
    return np.ascontiguousarray(pack.reshape(128, 8192))


def _build_nc():
    import concourse.tile as tile
    from concourse import bacc, mybir

    FP = mybir.dt.float32
    FH = mybir.dt.float16
    AF = mybir.ActivationFunctionType
    OP = mybir.AluOpType

    nc = bacc.Bacc("TRN2", target_bir_lowering=False, debug=False, num_devices=NCORES)

    p0_d = nc.dram_tensor("p0", [28, 16384], FH, kind="ExternalInput")
    wpk_d = nc.dram_tensor("wpk", [128, 1984], FH, kind="ExternalInput")
    bpk_d = nc.dram_tensor("bpk", [128, 16], FP, kind="ExternalInput")
    dw1_d = nc.dram_tensor("dw1k", [128, 8192], FH, kind="ExternalInput")
    out_d = nc.dram_tensor("out", [64, 10], FP, kind="ExternalOutput")

    with tile.TileContext(nc) as tc, ExitStack() as top:
        consts = top.enter_context(tc.tile_pool(name="consts", bufs=1))
        acts = top.enter_context(tc.tile_pool(name="acts", bufs=1))
        drams = top.enter_context(tc.tile_pool(name="drams", bufs=1, space="DRAM"))

        # ---- persistent tiles ----
        wpk = consts.tile([128, 1984], FH)
        bpk = consts.tile([128, 16], FP)
        dw1s = acts.tile([128, 8192], FH)
        l0out = acts.tile([128, 64 * 8 * 34], FH)  # [2x64ch, b, r(8), w(34)]
        l0v = l0out.rearrange("p (b r w) -> p b r w", b=64, r=8, w=34)
        l1out = acts.tile([128, 64 * 6 * 34], FH)  # [128ch, b, s(6), w(34)]
        l1v = l1out.rearrange("p (b s w) -> p b s w", b=64, s=6, w=34)
        l2out = acts.tile([128, 4 * 32 * 64], FH)  # [128ch, t(4), x(32), b(64)]
        l2va = l2out.rearrange("p (t x b) -> p b t x", t=4, x=32, b=64)
        y1 = consts.tile([64, 64], FP)

        # ---- constant DMAs: w0 slice first (tiny, unblocks L0) ----
        nc.scalar.dma_start(wpk[:, 0:64], wpk_d[:, 0:64])
        nc.scalar.dma_start(wpk[:, 64:1984], wpk_d[:, 64:1984])
        nc.scalar.dma_start(bpk[:], bpk_d[:, :])
        nc.gpsimd.dma_start(dw1s[:], dw1_d[:, :])

        # weight views into the packed tile
        w0s = wpk[0:28, 0:64]
        w1p = [wpk[:, 64 + 128 * ky : 192 + 128 * ky] for ky in range(3)]
        w1s = [wpk[0:64, 448 + 128 * ky : 576 + 128 * ky] for ky in range(3)]
        w2t = [wpk[:, 832 + 128 * t : 960 + 128 * t] for t in range(9)]
        b0c = bpk[0:64, 0:1]
        b1c = bpk[:, 1:2]
        bAc = bpk[:, 2:3]  # b1 - 1e9 on core 0 (zeroes L1 row s=0)
        bBc = bpk[:, 3:4]  # b1 - 1e9 on core 7 (zeroes L1 row s=5)
        b2c = bpk[:, 4:5]
        db1c = bpk[0:50, 5:6]
        dw2s = bpk[0:51, 6:16]

        # zero the w-pad columns (x=-1 / x=32); ones row for dense2 bias
        nc.vector.memset(l0v[0:64, :, :, 0:1], 0.0)
        nc.vector.memset(l0v[0:64, :, :, 33:34], 0.0)
        nc.vector.memset(l1v[:, :, :, 0:1], 0.0)
        nc.vector.memset(l1v[:, :, :, 33:34], 0.0)
        # rows 32:63 memset (32-aligned); ACT later overwrites 32:50, so
        # row 50 stays 1.0 as the dense2 bias row (rows 51:64 unused).
        nc.vector.memset(y1[32:64, :], 1.0)

        # ---- L0 + L1 pipelined per 8-image group ----
        with tc.tile_pool(name="p0pool", bufs=3) as p0pool, tc.tile_pool(
            name="psum0", bufs=2, space="PSUM"
        ) as psum0, tc.tile_pool(name="psum1", bufs=4, space="PSUM") as psum1:
            for g in range(8):
                p0t = p0pool.tile([28, 2048], FH, tag="p0t")
                nc.sync.dma_start(p0t[:], p0_d[:, 2048 * g : 2048 * (g + 1)])
                for u in range(4):
                    i = g * 4 + u  # image pair index
                    ps = psum0.tile([64, 512], FP, tag="ps0")
                    nc.tensor.matmul(
                        ps[:],
                        w0s,
                        p0t[:, 512 * u : 512 * (u + 1)],
                        start=True,
                        stop=True,
                    )
                    psv = ps.rearrange("p (b r w) -> p b r w", b=2, r=8, w=32)
                    nc.scalar.activation(
                        l0v[0:64, 2 * i : 2 * i + 2, :, 1:33],
                        psv[:, :, :, :],
                        AF.Relu,
                        bias=b0c,
                    )
                    # block1 = block0 shifted one x to the left (tap pairing)
                    nc.vector.tensor_copy(
                        l0v[64:128, 2 * i : 2 * i + 2, :, 0:33],
                        l0v[0:64, 2 * i : 2 * i + 2, :, 1:34],
                    )
                for jj in range(4):
                    j = g * 4 + jj
                    ps = psum1.tile([128, 384], FP, tag="ps1")
                    for ky in range(3):
                        nc.tensor.matmul(
                            ps[:],
                            w1p[ky],
                            l0v[0:128, 2 * j : 2 * j + 2, ky : ky + 6, 0:32],
                            start=(ky == 0),
                            stop=False,
                        )
                    for ky in range(3):
                        nc.tensor.matmul(
                            ps[:],
                            w1s[ky],
                            l0v[0:64, 2 * j : 2 * j + 2, ky : ky + 6, 2:34],
                            start=False,
                            stop=(ky == 2),
                        )
                    psv = ps.rearrange("p (b s w) -> p b s w", b=2, s=6, w=32)
                    # relu(x + b): bulk rows on DVE (gpsimd can't read
                    # PSUM); edge rows on ACT with the per-core poison
                    # bias so out-of-image halo rows become 0.
                    nc.vector.tensor_scalar(
                        l1v[:, 2 * j : 2 * j + 2, 1:5, 1:33],
                        psv[:, :, 1:5, :],
                        b1c,
                        0.0,
                        op0=OP.add,
                        op1=OP.max,
                    )
                    nc.scalar.activation(
                        l1v[:, 2 * j : 2 * j + 2, 0:1, 1:33],
                        psv[:, :, 0:1, :],
                        AF.Relu,
                        bias=bAc,
                    )
                    nc.scalar.activation(
                        l1v[:, 2 * j : 2 * j + 2, 5:6, 1:33],
                        psv[:, :, 5:6, :],
                        AF.Relu,
                        bias=bBc,
                    )

        # ---- L2 (dense 128->128 per tap) + dense1 + tail ----
        with tc.tile_pool(name="psum2", bufs=2, space="PSUM") as psum2, tc.tile_pool(
            name="psumd", bufs=1, space="PSUM"
        ) as psumd, tc.tile_pool(name="psume", bufs=1, space="PSUM") as psume:
            for q in range(16):
                ps = psum2.tile([128, 512], FP, tag="ps2")
                t = 0
                for ky in range(3):
                    for kx in range(3):
                        nc.tensor.matmul(
                            ps[:],
                            w2t[t],
                            l1v[:, 4 * q : 4 * q + 4, ky : ky + 4, kx : kx + 32],
                            start=(t == 0),
                            stop=(t == 8),
                        )
                        t += 1
                psv = ps.rearrange("p (b t x) -> p b t x", b=4, t=4, x=32)
                nc.scalar.activation(
                    l2va[:, 4 * q : 4 * q + 4, :, :],
                    psv[:, :, :, :],
                    AF.Relu,
                    bias=b2c,
                )

            # dense1 partial: 64 stacked-pair matmuls into psum [128, 128]
            # (stationary col kk*64+j, rows 50:64 of each half zero-padded
            # so the diagonal extraction reads at 32-aligned partitions)
            psd = psumd.tile([128, 128], FP)
            for m in range(64):
                nc.tensor.matmul(
                    psd[:],
                    dw1s[:, 128 * m : 128 * (m + 1)],
                    l2out[:, 128 * m : 128 * (m + 1)],
                    start=(m == 0),
                    stop=(m == 63),
                )
            ar_h = consts.tile([50, 64], FP)
            nc.vector.tensor_copy(ar_h[:], psd[64:114, 64:128])
            ar_s = consts.tile([50, 64], FP)
            nc.vector.tensor_tensor(ar_s[:], psd[0:50, 0:64], ar_h[:], op=OP.add)

            # ---- AllReduce the [50, 64] partial across the 8 cores ----
            in_b = drams.tile([50, 64], FP)
            out_b = drams.tile([50, 64], FP)
            nc.sync.dma_start(in_b[:], ar_s[:])
            nc.gpsimd.collective_compute(
                "AllReduce",
                OP.add,
                replica_groups=[list(range(NCORES))],
                ins=[in_b.opt()],
                outs=[out_b.opt()],
            )
            ar_o = consts.tile([50, 64], FP)
            nc.sync.dma_start(ar_o[:], out_b[:])

            # ---- dense2 + softmax (replicated tail) ----
            nc.scalar.activation(y1[0:50, :], ar_o[:], AF.Relu, bias=db1c)
            pse = psume.tile([64, 10], FP)
            nc.tensor.matmul(pse[:], y1[0:51, :], dw2s, start=True, stop=True)
            ex = consts.tile([64, 10], FP)
            sm = consts.tile([64, 1], FP)
            nc.scalar.activation(ex[:], pse[:], AF.Exp, accum_out=sm[:])
            rc = consts.tile([64, 1], FP)
            nc.vector.reciprocal(rc[:], sm[:])
            outs = consts.tile([64, 10], FP)
            nc.vector.tensor_scalar_mul(outs[:], ex[:], rc[:, 0:1])
            nc.sync.dma_start(out_d[:, :], outs[:])

    nc.finalize()
    return nc


def _get_nc():
    if "nc" not in _CACHE:
        _CACHE["nc"] = _build_nc()
    return _CACHE["nc"]


def _prep_in_maps(inputs, cw0, cb0, cw1, cb1, cw2, cb2, dw1, db1, dw2, db2):
    idx1, idx2 = _make_idx()
    w1d = _dense_w(np.asarray(cw1, np.float32), idx1, 64)
    w2d = _dense_w(np.asarray(cw2, np.float32), idx2, 128)

    wpk = np.zeros((128, 1984), np.float16)
    wpk[0:27, 0:64] = np.asarray(cw0, np.float32).reshape(27, 64)
    wpk[27, 0:64] = NEGH
    for ky in range(3):
        wpk[:, 64 + 128 * ky : 192 + 128 * ky] = np.concatenate(
            [w1d[ky, 0], w1d[ky, 1]], axis=0
        )
        wpk[0:64, 448 + 128 * ky : 576 + 128 * ky] = w1d[ky, 2]
    w2f = w2d.reshape(9, 128, 128)
    for t in range(9):
        wpk[:, 832 + 128 * t : 960 + 128 * t] = w2f[t]
    wpk = np.ascontiguousarray(wpk)

    cb0 = np.asarray(cb0, np.float32)
    cb1 = np.asarray(cb1, np.float32)
    cb2 = np.asarray(cb2, np.float32)
    db1 = np.asarray(db1, np.float32)
    dw2 = np.asarray(dw2, np.float32)
    db2 = np.asarray(db2, np.float32)

    x = np.asarray(inputs, np.float32).reshape(B, H, W, 3)
    dw1 = np.asarray(dw1, np.float32)
    in_maps = []
    for k in range(NCORES):
        bpk = np.zeros((128, 16), np.float32)
        bpk[0:64, 0] = cb0
        bpk[:, 1] = cb1
        bpk[:, 2] = cb1 + (NEG if k == 0 else 0.0)
        bpk[:, 3] = cb1 + (NEG if k == NCORES - 1 else 0.0)
        bpk[:, 4] = cb2
        bpk[0:50, 5] = db1
        bpk[0:50, 6:16] = dw2
        bpk[50, 6:16] = db2
        m = {
            "wpk": wpk,
            "bpk": np.ascontiguousarray(bpk),
            "p0": _build_p0(x, k),
            # [16384, 50] -> [c, pair m, kk, j] -> [c, m*128 + kk*64 + j]
            "dw1k": _pack_dw1(dw1, k),
        }
        in_maps.append(m)
    return in_maps


def _run(inputs_dict, trace=False):
    from concourse.bass_utils import run_bass_kernel_spmd

    nc = _get_nc()
    in_maps = _prep_in_maps(**inputs_dict)
    res = run_bass_kernel_spmd(
        nc, in_maps, core_ids=list(range(NCORES)), trace=trace
    )
    out = np.asarray(res.results[0]["out"], np.float32)
    return out, res


def kernel(**inputs):
    out, _ = _run(inputs, trace=False)
    return out
